# revision 27
# baseline (speedup 1.0000x reference)
"""Trainium2 Bass kernel for DifferentiableNeuralGas loss.

loss = mean(exp(-(soft_rank-1)/LAMBDA) * distances) over [N, K]
  distances[n,k] = ||data[n] - weights[k]||_2
  soft_rank[n,i] = 1 + sum_{j != i} sigmoid((d[n,i]-d[n,j])/TAU)

Key identities used on-device:
  S[n,i] := sum_{all j} sigmoid((d_i-d_j)/TAU)   (includes j==i term = 0.5)
  soft_rank - 1 = S - 0.5
  neighborhood = exp(-(S - 0.5)/LAMBDA) = exp(S*(-1/LAMBDA) + 1/(2*LAMBDA))

Per-core pipeline (data rows sharded 8 ways, weights replicated):
  A) distances (transposed) D_tT[k,n] = sqrt(w2[k] + x2[n] - 2*w.x) via PE
     matmuls + ACT sqrt with per-partition bias.
  B) ARG[n,(i,j)] = d[n,i]-d[n,j] built by ONE PE matmul per 512-slice
     against a constant selection matrix msel[c,(i,j)] = I[c,i]-I[c,j];
     giant ACT sigmoid instructions (PSUM->SBUF); DVE grouped reduce over j.
  C) neighborhood+product: ACT exp, fused DVE multiply-accumulate against
     PE-transposed distances, partition reduce via ones-matmul.
Host sums the 8 per-core partial sums and divides by N*K.
"""

import sys

sys.path.insert(0, "/opt/trn_rl_repo")

from contextlib import ExitStack

import numpy as np

import concourse.bass as bass
import concourse.mybir as mybir
import concourse.tile as tile
from concourse import bacc
from concourse.bass_utils import run_bass_kernel_spmd


def _install_ntff_hook():
    """The agent image's antenv lacks axon_hooks, so trn_boot's NTFF
    profile hook never registers; recreate the tiny registry here so
    trace=True can capture HW profiles through libaxon_pjrt."""
    import types

    if "antenv.axon_hooks" in sys.modules:
        return
    mod = types.ModuleType("antenv.axon_hooks")
    _hook = [None]
    mod.set_axon_ntff_profile_hook = lambda h: _hook.__setitem__(0, h)
    mod.get_axon_ntff_profile_hook = lambda: _hook[0]
    sys.modules["antenv.axon_hooks"] = mod
    try:
        import trn_agent_boot.trn_boot as tb

        mod.set_axon_ntff_profile_hook(
            tb._ntff_profile_via_ctypes("/opt/axon/libaxon_pjrt.so"))
    except Exception:
        pass


_install_ntff_hook()

F32 = mybir.dt.float32
F16 = mybir.dt.float16
AF = mybir.ActivationFunctionType
ALU = mybir.AluOpType
AX = mybir.AxisListType

N, D, K = 16384, 64, 128
NCORES = 8
TAU = 0.2
LAMBDA = 8.0
P = 128
CH = 2048            # free-dim elements per sigmoid chunk
MM = 512             # fp32 moving-operand max per matmul
GRP = CH // K        # i-values per chunk
USE_HILO = False     # exact fp16 hi/lo ARG matmuls (2x PE) vs single fp16


def build(nloc: int) -> bass.Bass:
    nt = nloc // P
    nch = (K * K) // CH

    nc = bacc.Bacc()
    x_d = nc.dram_tensor("x", [nloc, D], F32, kind="ExternalInput")
    w_d = nc.dram_tensor("w", [K, D], F32, kind="ExternalInput")
    msel_d = nc.dram_tensor("msel", [K, K * K], F16, kind="ExternalInput")
    ident_d = nc.dram_tensor("ident", [P, P], F32, kind="ExternalInput")
    out_d = nc.dram_tensor("out", [1, 1], F32, kind="ExternalOutput")

    with ExitStack() as ctx:
        tc = ctx.enter_context(tile.TileContext(nc))
        singles = ctx.enter_context(tc.tile_pool(name="singles", bufs=1))

        ident_sb = singles.tile([P, P], F32, tag="ident")
        nc.sync.dma_start(out=ident_sb, in_=ident_d[:, :])
        w_sb = singles.tile([K, D], F32, tag="w")
        nc.sync.dma_start(out=w_sb, in_=w_d[:, :])
        ones64 = singles.tile([D, P], F32, tag="ones64")
        nc.vector.memset(ones64, 1.0)
        ones128 = singles.tile([P, 1], F32, tag="ones128")
        nc.vector.memset(ones128, 1.0)
        expbias = singles.tile([P, 1], F32, tag="expbias")
        nc.vector.memset(expbias, 1.0 / (2.0 * LAMBDA))

        # w2[k] = sum_d w[k,d]^2  (per-partition column, bias for sqrt)
        scr_w = singles.tile([K, D], F32, tag="scrw")
        w2col = singles.tile([K, 1], F32, tag="w2col")
        nc.vector.scalar_tensor_tensor(
            out=scr_w, in0=w_sb, scalar=1.0, in1=w_sb,
            op0=ALU.bypass, op1=ALU.mult, accum_out=w2col)

        dtt = {}   # t -> D_tT tile [K=128 part (cluster), P free (row)]
        dhi = {}
        dlo = {}
        S_tiles = {}

        # ---------------- phase A: distances ----------------
        # d2 for all tiles gathered into one SBUF slab -> ONE sqrt, ONE cast
        d2_all = singles.tile([K, nt * P], F32, tag="d2_all")
        D_all = singles.tile([K, nt * P], F32, tag="D_all")
        Dhi_all = singles.tile([K, nt * P], F16, tag="Dhi_all")
        with tc.tile_pool(name="psumA", bufs=2, space="PSUM") as psumA, \
             tc.tile_pool(name="tmpA", bufs=4) as tmpA:
            psum_wT = psumA.tile([D, K], F32, tag="wT")
            nc.tensor.transpose(psum_wT, w_sb, ident_sb)
            wT_m2 = singles.tile([D, K], F32, tag="wTm2")
            nc.vector.tensor_scalar_mul(wT_m2, psum_wT, -2.0)

            for t in range(nt):
                x_t = tmpA.tile([P, D], F32, tag="x")
                nc.sync.dma_start(out=x_t, in_=x_d[t * P:(t + 1) * P, :])
                psum_xT = psumA.tile([D, P], F32, tag="xT")
                nc.tensor.transpose(psum_xT, x_t, ident_sb)
                xT = tmpA.tile([D, P], F32, tag="xTsb")
                nc.vector.tensor_copy(xT, psum_xT)
                xsqT = tmpA.tile([D, P], F32, tag="xsqT")
                nc.vector.scalar_tensor_tensor(
                    out=xsqT, in0=xT, scalar=1.0, in1=xT,
                    op0=ALU.bypass, op1=ALU.mult)

                psum_dT = psumA.tile([K, P], F32, tag="dT")
                nc.tensor.matmul(psum_dT, wT_m2, xT, start=True, stop=False)
                nc.tensor.matmul(psum_dT, ones64, xsqT,
                                 start=False, stop=True)
                nc.vector.tensor_copy(d2_all[:, t * P:(t + 1) * P], psum_dT)

        nc.scalar.activation(D_all, d2_all, AF.Sqrt, bias=w2col, scale=1.0)
        nc.vector.tensor_copy(Dhi_all, D_all)
        for t in range(nt):
            dtt[t] = D_all[:, t * P:(t + 1) * P]
            dhi[t] = Dhi_all[:, t * P:(t + 1) * P]
        if USE_HILO:
            Dlo_all = singles.tile([K, nt * P], F16, tag="Dlo_all")
            nc.vector.scalar_tensor_tensor(
                out=Dlo_all, in0=D_all, scalar=1.0, in1=Dhi_all,
                op0=ALU.bypass, op1=ALU.subtract)
            for t in range(nt):
                dlo[t] = Dlo_all[:, t * P:(t + 1) * P]

        # msel DMAs issued after phase A's so x tiles aren't queued
        # behind 4MB of constants (they finish well before phase B needs them)
        msel_sb = []
        for c in range(nch):
            m = singles.tile([P, CH], F16, tag=f"msel{c}")
            nc.sync.dma_start(out=m, in_=msel_d[:, c * CH:(c + 1) * CH])
            msel_sb.append(m)

        # ---------------- phase B: sigmoid rank sums ----------------
        with tc.tile_pool(name="psumB", bufs=2, space="PSUM") as psumB, \
             tc.tile_pool(name="sigp", bufs=3) as sigp, \
             tc.tile_pool(name="foldp", bufs=2) as foldp:
            S_all = singles.tile([P, nt * K], F32, tag="S_all")
            for t in range(nt):
                S_t = S_all[:, t * K:(t + 1) * K]
                S_tiles[t] = S_t
                f1 = foldp.tile([P, K * K // 2], F16, tag="f1")
                for c in range(nch):
                    psum_arg = psumB.tile([P, CH], F32, tag="arg")
                    nmm = CH // MM
                    for m in range(nmm):
                        nc.tensor.matmul(
                            psum_arg[:, m * MM:(m + 1) * MM],
                            dhi[t],
                            msel_sb[c][:, m * MM:(m + 1) * MM],
                            start=True, stop=not USE_HILO,
                            skip_group_check=True)
                    if USE_HILO:
                        for m in range(nmm):
                            nc.tensor.matmul(
                                psum_arg[:, m * MM:(m + 1) * MM],
                                dlo[t],
                                msel_sb[c][:, m * MM:(m + 1) * MM],
                                start=False, stop=True, skip_group_check=True)
                    sig = sigp.tile([P, CH], F16, tag="sig")
                    nc.scalar.activation(sig, psum_arg, AF.Sigmoid,
                                         bias=0.0, scale=1.0 / TAU)
                    # per-chunk fold1 over j (fp16 2x DVE mode)
                    s3 = sig[:].rearrange("p (i j) -> p i j", j=K)
                    nc.vector.tensor_tensor(
                        out=f1[:, c * CH // 2:(c + 1) * CH // 2].rearrange(
                            "p (i j) -> p i j", j=K // 2),
                        in0=s3[:, :, 0:K // 2], in1=s3[:, :, K // 2:K],
                        op=ALU.add)
                # tile-level fold2/fold3 + grouped reduce
                f1v = f1[:].rearrange("p (i j) -> p i j", j=K // 2)
                f2 = foldp.tile([P, K * K // 4], F16, tag="f2")
                nc.vector.tensor_tensor(
                    out=f2[:].rearrange("p (i j) -> p i j", j=K // 4),
                    in0=f1v[:, :, 0:K // 4], in1=f1v[:, :, K // 4:K // 2],
                    op=ALU.add)
                f2v = f2[:].rearrange("p (i j) -> p i j", j=K // 4)
                f3 = foldp.tile([P, K * K // 8], F16, tag="f3")
                nc.vector.tensor_tensor(
                    out=f3[:].rearrange("p (i j) -> p i j", j=K // 8),
                    in0=f2v[:, :, 0:K // 8], in1=f2v[:, :, K // 8:K // 4],
                    op=ALU.add)
                nc.vector.reduce_sum(
                    out=S_t,
                    in_=f3[:].rearrange("p (i j) -> p i j", j=K // 8),
                    axis=AX.X)

        # ---------------- phase C: neighborhood * distance ----------------
        with tc.tile_pool(name="psumC", bufs=3, space="PSUM") as psumC, \
             tc.tile_pool(name="tmpC", bufs=3) as tmpC:
            losscol = singles.tile([P, nt], F32, tag="losscol")
            E_all = singles.tile([P, nt * K], F32, tag="E_all")
            nc.scalar.activation(E_all, S_all, AF.Exp,
                                 bias=expbias, scale=-1.0 / LAMBDA)
            for t in range(nt):
                psum_Dt = psumC.tile([P, K], F32, tag="Dt")
                nc.tensor.transpose(psum_Dt, dtt[t], ident_sb)
                scr = tmpC.tile([P, K], F32, tag="scrC")
                nc.vector.scalar_tensor_tensor(
                    out=scr, in0=E_all[:, t * K:(t + 1) * K], scalar=1.0,
                    in1=psum_Dt, op0=ALU.bypass, op1=ALU.mult,
                    accum_out=losscol[:, t:t + 1])
            losssum = singles.tile([P, 1], F32, tag="losssum")
            nc.vector.reduce_sum(out=losssum, in_=losscol, axis=AX.X)
            psum_fin = psumC.tile([1, 1], F32, tag="fin")
            nc.tensor.matmul(psum_fin, losssum, ones128, start=True, stop=True)
            out_sb = singles.tile([1, 1], F32, tag="outsb")
            nc.vector.tensor_copy(out_sb, psum_fin)
            nc.sync.dma_start(out=out_d[:, :], in_=out_sb)

    nc.finalize()
    return nc


def make_msel() -> np.ndarray:
    I = np.eye(K, dtype=np.float32)
    return np.ascontiguousarray(
        (I[:, :, None] - I[:, None, :]).reshape(K, K * K).astype(np.float16))


_BUILT: dict[int, bass.Bass] = {}


def get_built(nloc: int) -> bass.Bass:
    if nloc not in _BUILT:
        _BUILT[nloc] = build(nloc)
    return _BUILT[nloc]


def make_in_maps(data: np.ndarray, weights: np.ndarray, ncores: int):
    nloc = data.shape[0] // ncores
    msel = make_msel()
    ident = np.eye(P, dtype=np.float32)
    return [
        {
            "x": np.ascontiguousarray(data[c * nloc:(c + 1) * nloc]),
            "w": np.ascontiguousarray(weights),
            "msel": msel,
            "ident": ident,
        }
        for c in range(ncores)
    ]


def run(data, weights, trace: bool = False):
    """Returns (loss, BassKernelResults)."""
    data = np.ascontiguousarray(np.asarray(data, dtype=np.float32))
    weights = np.ascontiguousarray(np.asarray(weights, dtype=np.float32))
    n, k = data.shape[0], weights.shape[0]
    nloc = n // NCORES
    nc = get_built(nloc)
    in_maps = make_in_maps(data, weights, NCORES)
    res = run_bass_kernel_spmd(nc, in_maps, list(range(NCORES)), trace=trace)
    total = sum(float(r["out"][0, 0]) for r in res.results)
    loss = np.float32(total / (n * k))
    return loss, res


def kernel(data, weights):
    loss, _ = run(data, weights)
    return loss


# revision 30
# speedup vs baseline: 1.0300x; 1.0300x over previous
"""Trainium2 Bass kernel for DifferentiableNeuralGas loss.

loss = mean(exp(-(soft_rank-1)/LAMBDA) * distances) over [N, K]
  distances[n,k] = ||data[n] - weights[k]||_2
  soft_rank[n,i] = 1 + sum_{j != i} sigmoid((d[n,i]-d[n,j])/TAU)

Key identities used on-device:
  S[n,i] := sum_{all j} sigmoid((d_i-d_j)/TAU)   (includes j==i term = 0.5)
  soft_rank - 1 = S - 0.5
  neighborhood = exp(-(S - 0.5)/LAMBDA) = exp(S*(-1/LAMBDA) + 1/(2*LAMBDA))

Per-core pipeline (data rows sharded 8 ways, weights replicated):
  A) distances (transposed) D_tT[k,n] = sqrt(w2[k] + x2[n] - 2*w.x) via PE
     matmuls + ACT sqrt with per-partition bias.
  B) ARG[n,(i,j)] = d[n,i]-d[n,j] built by ONE PE matmul per 512-slice
     against a constant selection matrix msel[c,(i,j)] = I[c,i]-I[c,j];
     giant ACT sigmoid instructions (PSUM->SBUF); DVE grouped reduce over j.
  C) neighborhood+product: ACT exp, fused DVE multiply-accumulate against
     PE-transposed distances, partition reduce via ones-matmul.
Host sums the 8 per-core partial sums and divides by N*K.
"""

import sys

sys.path.insert(0, "/opt/trn_rl_repo")

from contextlib import ExitStack

import numpy as np

import concourse.bass as bass
import concourse.mybir as mybir
import concourse.tile as tile
from concourse import bacc
from concourse.bass_utils import run_bass_kernel_spmd


def _install_ntff_hook():
    """The agent image's antenv lacks axon_hooks, so trn_boot's NTFF
    profile hook never registers; recreate the tiny registry here so
    trace=True can capture HW profiles through libaxon_pjrt."""
    import types

    if "antenv.axon_hooks" in sys.modules:
        return
    mod = types.ModuleType("antenv.axon_hooks")
    _hook = [None]
    mod.set_axon_ntff_profile_hook = lambda h: _hook.__setitem__(0, h)
    mod.get_axon_ntff_profile_hook = lambda: _hook[0]
    sys.modules["antenv.axon_hooks"] = mod
    try:
        import trn_agent_boot.trn_boot as tb

        mod.set_axon_ntff_profile_hook(
            tb._ntff_profile_via_ctypes("/opt/axon/libaxon_pjrt.so"))
    except Exception:
        pass


_install_ntff_hook()

F32 = mybir.dt.float32
F16 = mybir.dt.float16
AF = mybir.ActivationFunctionType
ALU = mybir.AluOpType
AX = mybir.AxisListType

N, D, K = 16384, 64, 128
NCORES = 8
TAU = 0.2
LAMBDA = 8.0
P = 128
CH = 2048            # free-dim elements per sigmoid chunk
MM = 512             # fp32 moving-operand max per matmul
GRP = CH // K        # i-values per chunk
USE_HILO = False     # exact fp16 hi/lo ARG matmuls (2x PE) vs single fp16


def build(nloc: int) -> bass.Bass:
    nt = nloc // P
    nch = (K * K) // CH

    nc = bacc.Bacc()
    xT_d = nc.dram_tensor("xT", [D, nloc], F32, kind="ExternalInput")
    w_d = nc.dram_tensor("w", [K, D], F32, kind="ExternalInput")
    msel_d = nc.dram_tensor("msel", [K, K * K], F16, kind="ExternalInput")
    ident_d = nc.dram_tensor("ident", [P, P], F32, kind="ExternalInput")
    out_d = nc.dram_tensor("out", [1, 1], F32, kind="ExternalOutput")

    with ExitStack() as ctx:
        tc = ctx.enter_context(tile.TileContext(nc))
        singles = ctx.enter_context(tc.tile_pool(name="singles", bufs=1))

        ident_sb = singles.tile([P, P], F32, tag="ident")
        nc.sync.dma_start(out=ident_sb, in_=ident_d[:, :])
        w_sb = singles.tile([K, D], F32, tag="w")
        nc.sync.dma_start(out=w_sb, in_=w_d[:, :])
        ones64 = singles.tile([D, P], F32, tag="ones64")
        nc.vector.memset(ones64, 1.0)
        ones128 = singles.tile([P, 1], F32, tag="ones128")
        nc.vector.memset(ones128, 1.0)
        expbias = singles.tile([P, 1], F32, tag="expbias")
        nc.vector.memset(expbias, 1.0 / (2.0 * LAMBDA))

        # w2[k] = sum_d w[k,d]^2  (per-partition column, bias for sqrt)
        scr_w = singles.tile([K, D], F32, tag="scrw")
        w2col = singles.tile([K, 1], F32, tag="w2col")
        nc.vector.scalar_tensor_tensor(
            out=scr_w, in0=w_sb, scalar=1.0, in1=w_sb,
            op0=ALU.bypass, op1=ALU.mult, accum_out=w2col)

        dtt = {}   # t -> D_tT tile [K=128 part (cluster), P free (row)]
        dhi = {}
        dlo = {}
        S_tiles = {}

        # ---------------- phase A: distances ----------------
        # d2 for all tiles gathered into one SBUF slab -> ONE sqrt, ONE cast
        d2_all = singles.tile([K, nt * P], F32, tag="d2_all")
        D_all = singles.tile([K, nt * P], F32, tag="D_all")
        Dhi_all = singles.tile([K, nt * P], F16, tag="Dhi_all")
        with tc.tile_pool(name="psumA", bufs=2, space="PSUM") as psumA:
            psum_wT = psumA.tile([D, K], F32, tag="wT")
            nc.tensor.transpose(psum_wT, w_sb, ident_sb)
            wT_m2 = singles.tile([D, K], F32, tag="wTm2")
            nc.vector.tensor_scalar_mul(wT_m2, psum_wT, -2.0)

            xT_all = singles.tile([D, nloc], F32, tag="xT_all")
            BB = 512
            for b in range(nloc // BB):
                nc.sync.dma_start(out=xT_all[:, b * BB:(b + 1) * BB],
                                  in_=xT_d[:, b * BB:(b + 1) * BB])
            xsq_all = singles.tile([D, nloc], F32, tag="xsq_all")
            for b in range(nloc // BB):
                sl = slice(b * BB, (b + 1) * BB)
                nc.vector.scalar_tensor_tensor(
                    out=xsq_all[:, sl], in0=xT_all[:, sl], scalar=1.0,
                    in1=xT_all[:, sl], op0=ALU.bypass, op1=ALU.mult)
                psum_dT = psumA.tile([K, BB], F32, tag="dT")
                nc.tensor.matmul(psum_dT, wT_m2, xT_all[:, sl],
                                 start=True, stop=False)
                nc.tensor.matmul(psum_dT, ones64, xsq_all[:, sl],
                                 start=False, stop=True)
                nc.vector.tensor_copy(d2_all[:, sl], psum_dT)

        nc.scalar.activation(D_all, d2_all, AF.Sqrt, bias=w2col, scale=1.0)
        nc.vector.tensor_copy(Dhi_all, D_all)
        for t in range(nt):
            dtt[t] = D_all[:, t * P:(t + 1) * P]
            dhi[t] = Dhi_all[:, t * P:(t + 1) * P]
        if USE_HILO:
            Dlo_all = singles.tile([K, nt * P], F16, tag="Dlo_all")
            nc.vector.scalar_tensor_tensor(
                out=Dlo_all, in0=D_all, scalar=1.0, in1=Dhi_all,
                op0=ALU.bypass, op1=ALU.subtract)
            for t in range(nt):
                dlo[t] = Dlo_all[:, t * P:(t + 1) * P]

        # msel DMAs issued after phase A's so x tiles aren't queued
        # behind 4MB of constants (they finish well before phase B needs them)
        msel_sb = []
        for c in range(nch):
            m = singles.tile([P, CH], F16, tag=f"msel{c}")
            nc.sync.dma_start(out=m, in_=msel_d[:, c * CH:(c + 1) * CH])
            msel_sb.append(m)

        # ---------------- phase B: sigmoid rank sums ----------------
        with tc.tile_pool(name="psumB", bufs=2, space="PSUM") as psumB, \
             tc.tile_pool(name="sigp", bufs=3) as sigp, \
             tc.tile_pool(name="foldp", bufs=2) as foldp:
            S_all = singles.tile([P, nt * K], F32, tag="S_all")
            for t in range(nt):
                S_t = S_all[:, t * K:(t + 1) * K]
                S_tiles[t] = S_t
                f1 = foldp.tile([P, K * K // 2], F16, tag="f1")
                for c in range(nch):
                    psum_arg = psumB.tile([P, CH], F32, tag="arg")
                    nmm = CH // MM
                    for m in range(nmm):
                        nc.tensor.matmul(
                            psum_arg[:, m * MM:(m + 1) * MM],
                            dhi[t],
                            msel_sb[c][:, m * MM:(m + 1) * MM],
                            start=True, stop=not USE_HILO,
                            skip_group_check=True)
                    if USE_HILO:
                        for m in range(nmm):
                            nc.tensor.matmul(
                                psum_arg[:, m * MM:(m + 1) * MM],
                                dlo[t],
                                msel_sb[c][:, m * MM:(m + 1) * MM],
                                start=False, stop=True, skip_group_check=True)
                    sig = sigp.tile([P, CH], F16, tag="sig")
                    nc.scalar.activation(sig, psum_arg, AF.Sigmoid,
                                         bias=0.0, scale=1.0 / TAU)
                    # per-chunk fold1 over j (fp16 2x DVE mode)
                    s3 = sig[:].rearrange("p (i j) -> p i j", j=K)
                    nc.vector.tensor_tensor(
                        out=f1[:, c * CH // 2:(c + 1) * CH // 2].rearrange(
                            "p (i j) -> p i j", j=K // 2),
                        in0=s3[:, :, 0:K // 2], in1=s3[:, :, K // 2:K],
                        op=ALU.add)
                # tile-level fold2/fold3 + grouped reduce
                f1v = f1[:].rearrange("p (i j) -> p i j", j=K // 2)
                f2 = foldp.tile([P, K * K // 4], F16, tag="f2")
                nc.vector.tensor_tensor(
                    out=f2[:].rearrange("p (i j) -> p i j", j=K // 4),
                    in0=f1v[:, :, 0:K // 4], in1=f1v[:, :, K // 4:K // 2],
                    op=ALU.add)
                f2v = f2[:].rearrange("p (i j) -> p i j", j=K // 4)
                f3 = foldp.tile([P, K * K // 8], F16, tag="f3")
                nc.vector.tensor_tensor(
                    out=f3[:].rearrange("p (i j) -> p i j", j=K // 8),
                    in0=f2v[:, :, 0:K // 8], in1=f2v[:, :, K // 8:K // 4],
                    op=ALU.add)
                nc.vector.reduce_sum(
                    out=S_t,
                    in_=f3[:].rearrange("p (i j) -> p i j", j=K // 8),
                    axis=AX.X)

        # ---------------- phase C: neighborhood * distance ----------------
        with tc.tile_pool(name="psumC", bufs=3, space="PSUM") as psumC, \
             tc.tile_pool(name="tmpC", bufs=3) as tmpC:
            losscol = singles.tile([P, nt], F32, tag="losscol")
            E_all = singles.tile([P, nt * K], F32, tag="E_all")
            nc.scalar.activation(E_all, S_all, AF.Exp,
                                 bias=expbias, scale=-1.0 / LAMBDA)
            for t in range(nt):
                psum_Dt = psumC.tile([P, K], F32, tag="Dt")
                nc.tensor.transpose(psum_Dt, dtt[t], ident_sb)
                scr = tmpC.tile([P, K], F32, tag="scrC")
                nc.vector.scalar_tensor_tensor(
                    out=scr, in0=E_all[:, t * K:(t + 1) * K], scalar=1.0,
                    in1=psum_Dt, op0=ALU.bypass, op1=ALU.mult,
                    accum_out=losscol[:, t:t + 1])
            losssum = singles.tile([P, 1], F32, tag="losssum")
            nc.vector.reduce_sum(out=losssum, in_=losscol, axis=AX.X)
            psum_fin = psumC.tile([1, 1], F32, tag="fin")
            nc.tensor.matmul(psum_fin, losssum, ones128, start=True, stop=True)
            out_sb = singles.tile([1, 1], F32, tag="outsb")
            nc.vector.tensor_copy(out_sb, psum_fin)
            nc.sync.dma_start(out=out_d[:, :], in_=out_sb)

    nc.finalize()
    return nc


def make_msel() -> np.ndarray:
    I = np.eye(K, dtype=np.float32)
    return np.ascontiguousarray(
        (I[:, :, None] - I[:, None, :]).reshape(K, K * K).astype(np.float16))


_BUILT: dict[int, bass.Bass] = {}


def get_built(nloc: int) -> bass.Bass:
    if nloc not in _BUILT:
        _BUILT[nloc] = build(nloc)
    return _BUILT[nloc]


def make_in_maps(data: np.ndarray, weights: np.ndarray, ncores: int):
    nloc = data.shape[0] // ncores
    msel = make_msel()
    ident = np.eye(P, dtype=np.float32)
    return [
        {
            "xT": np.ascontiguousarray(data[c * nloc:(c + 1) * nloc].T),
            "w": np.ascontiguousarray(weights),
            "msel": msel,
            "ident": ident,
        }
        for c in range(ncores)
    ]


def run(data, weights, trace: bool = False):
    """Returns (loss, BassKernelResults)."""
    data = np.ascontiguousarray(np.asarray(data, dtype=np.float32))
    weights = np.ascontiguousarray(np.asarray(weights, dtype=np.float32))
    n, k = data.shape[0], weights.shape[0]
    nloc = n // NCORES
    nc = get_built(nloc)
    in_maps = make_in_maps(data, weights, NCORES)
    res = run_bass_kernel_spmd(nc, in_maps, list(range(NCORES)), trace=trace)
    total = sum(float(r["out"][0, 0]) for r in res.results)
    loss = np.float32(total / (n * k))
    return loss, res


def kernel(data, weights):
    loss, _ = run(data, weights)
    return loss
